# revision 1
# baseline (speedup 1.0000x reference)
"""GatedGCN Trainium2 kernel — 8-core SPMD, self-contained.

Strategy
--------
dst-shard the graph across 8 NeuronCores. Node features live in DRAM as an
fp16 table with 4 nodes packed per 256B row, so the MoE bulk `dma_gather`
(int16 indices, 256B elements) can fetch `h[src]` for every edge in one
index window. Nodes are relabeled per shard by descending in-degree and
grouped into 128-node panels; each edge occupies a (node, slot) cell, so
the weighted segment-sum becomes a regular DVE multiply+reduce over the
panel (no scatter anywhere). Per-slot weights are expanded into 4 "planes"
(weight on the src%4 lane, 0 elsewhere), which also performs the 4-way
sub-row selection of the packed gather rows. Layer hand-off between cores
is a single fp16 AllGather. GRU / linear algebra runs feature-major with a
fused bias row (K=33 matmuls).
"""
import sys

sys.path.insert(0, "/opt/trn_rl_repo")

import numpy as np

import concourse.bacc as bacc
import concourse.bass as bass
import concourse.mybir as mybir
import concourse.tile as tile
from concourse.bass_utils import run_bass_kernel_spmd
from concourse.masks import make_identity

N = 100000
E = 1600000
H = 32
NCLS = 2
LAYERS = 2
NCORES = 8
KCH = 8  # idxs per dma_gather call = 128*KCH (SWDGE ring cap is ~65-72 descs)

F32 = mybir.dt.float32
F16 = mybir.dt.float16
I16 = mybir.dt.int16
AF = mybir.ActivationFunctionType
ALU = mybir.AluOpType


def _split_multiwaits(nc, max_waits=1):
    """This walrus build rejects >1 sync-wait per instruction; split extras
    onto same-engine InstNoOp predecessors (semantically identical)."""
    ctr = 0
    for fn in nc.m.functions:
        for bb in fn.blocks:
            new_insts = []
            for inst in bb.instructions:
                si = inst.sync_info
                waits = list(si.on_wait) if si is not None and si.on_wait else []
                if len(waits) > max_waits:
                    head, tail = waits[:-max_waits], waits[-max_waits:]
                    for i in range(0, len(head), max_waits):
                        ctr += 1
                        nop = mybir.InstNoOp(name=f"WSPLIT-{ctr}", engine=inst.engine)
                        nop.sync_info = mybir.SyncInfo(
                            on_wait=head[i : i + max_waits], on_update=[]
                        )
                        nc.register_instruction(nop, overwrite=True)
                        new_insts.append(nop)
                    inst.sync_info = mybir.SyncInfo(
                        on_wait=tail,
                        on_update=list(si.on_update) if si.on_update else [],
                    )
                new_insts.append(inst)
            bb.instructions[:] = new_insts


def _sizes(n):
    shard = n // NCORES
    shard_pad = -(-shard // 512) * 512
    panels = shard_pad // 128
    tabrows = NCORES * shard_pad // 4
    return shard, shard_pad, panels, tabrows


def _preprocess(edge_index, edge_weight):
    shard, shard_pad, panels, tabrows = _sizes(N)
    src_ = np.asarray(edge_index[0], dtype=np.int64)
    dst = np.asarray(edge_index[1], dtype=np.int64)
    src = src_
    w = np.asarray(edge_weight, dtype=np.float32)

    deg = np.bincount(dst, minlength=N)
    shards = np.arange(N) // shard
    order = np.lexsort((np.arange(N), -deg, shards))  # old ids by (shard, -deg)
    new_of_old = np.empty(N, dtype=np.int64)
    pos = np.arange(N)
    c_of_pos = pos // shard
    r_of_pos = pos - c_of_pos * shard
    new_of_old[order] = c_of_pos * shard_pad + r_of_pos

    s_new = new_of_old[src]
    d_new = new_of_old[dst]
    core = d_new // shard_pad
    r = d_new % shard_pad
    # slot index k per edge: occurrence number among edges sharing the dst
    eorder = np.argsort(d_new, kind="stable")
    ds = d_new[eorder]
    starts = np.r_[0, np.nonzero(np.diff(ds))[0] + 1]
    counts = np.diff(np.r_[starts, len(ds)])
    k_sorted = np.arange(len(ds)) - np.repeat(starts, counts)
    k = np.empty(src_.size, dtype=np.int64)
    k[eorder] = k_sorted

    # per-core per-panel K, unified across cores (SPMD: one program)
    deg_new = np.zeros(NCORES * shard_pad, dtype=np.int64)
    deg_new[new_of_old] = deg
    K_uni = np.zeros(panels, dtype=np.int64)
    for c in range(NCORES):
        base = c * shard_pad
        firsts = deg_new[base : base + shard_pad : 128]  # max of each panel
        K_uni = np.maximum(K_uni, firsts)
    K_uni = K_uni.astype(np.int64)
    sumK = int(K_uni.sum())
    col0 = np.zeros(panels + 1, dtype=np.int64)
    col0[1:] = np.cumsum(128 * K_uni)
    slots_total = int(col0[-1])

    p_of_edge = r // 128
    q_of_edge = r % 128
    slotpos = col0[p_of_edge] + k * 128 + q_of_edge

    idx_imgs = np.zeros((NCORES, 128, 8 * sumK), dtype=np.int16)
    w4_imgs = np.zeros((NCORES, 128, 4 * sumK), dtype=np.float16)
    for c in range(NCORES):
        m = core == c
        ia = np.zeros(slots_total, dtype=np.int16)
        wa = np.zeros(slots_total * 4, dtype=np.float16)
        ia[slotpos[m]] = (s_new[m] >> 2).astype(np.int16)
        wa[slotpos[m] * 4 + (s_new[m] & 3)] = w[m].astype(np.float16)
        icols = 0
        wcols = 0
        for p in range(panels):
            K = int(K_uni[p])
            if K == 0:
                continue
            a, b = int(col0[p]), int(col0[p + 1])
            blk = ia[a:b].reshape(K * 8, 16).T  # [16, 8K]
            idx_imgs[c, :, icols : icols + 8 * K] = np.tile(blk, (8, 1))
            wb = wa[4 * a : 4 * b].reshape(K, 128, 4)
            w4_imgs[c, :, wcols : wcols + 4 * K] = wb.transpose(1, 0, 2).reshape(
                128, 4 * K
            )
            icols += 8 * K
            wcols += 4 * K
    return {
        "order": order,
        "K_uni": K_uni,
        "idx_imgs": idx_imgs,
        "w4_imgs": w4_imgs,
        "sumK": sumK,
    }


def _w33(WT, b):
    """[K_in, K_out] weight^T stacked with bias row -> [K_in+1, K_out] f32."""
    return np.concatenate(
        [np.asarray(WT, np.float32), np.asarray(b, np.float32)[None, :]], axis=0
    )


_BUILD_CACHE = {}


def _build(K_uni, fuse):
    key = (tuple(int(x) for x in K_uni), tuple(float(x) for x in fuse))
    if key in _BUILD_CACHE:
        return _BUILD_CACHE[key]

    shard, shard_pad, panels, tabrows = _sizes(N)
    sumK = int(np.sum(K_uni))
    batches = panels // 4

    nc = bacc.Bacc(
        "TRN2",
        target_bir_lowering=False,
        debug=False,
        num_devices=NCORES,
        num_swdge_queues=4,
        dynamic_dma_scratch_size=8192,
    )
    xT = nc.dram_tensor("xT", [H + 1, shard_pad], F32, kind="ExternalInput").ap()
    idx_d = nc.dram_tensor("idx", [128, 8 * sumK], I16, kind="ExternalInput").ap()
    w4_d = nc.dram_tensor("w4", [128, 4 * sumK], F16, kind="ExternalInput").ap()
    w1_d = nc.dram_tensor("w1", [H + 1, H], F32, kind="ExternalInput").ap()
    wnn_d = nc.dram_tensor("wnn", [LAYERS * (H + 1), H], F32, kind="ExternalInput").ap()
    wih_d = nc.dram_tensor("wih", [H + 1, 3 * H], F32, kind="ExternalInput").ap()
    whh_d = nc.dram_tensor("whh", [H + 1, 3 * H], F32, kind="ExternalInput").ap()
    wout_d = nc.dram_tensor("wout", [H + 1, NCLS], F32, kind="ExternalInput").ap()
    out_d = nc.dram_tensor("out", [128, NCLS * panels], F32, kind="ExternalOutput").ap()

    shard_buf = nc.dram_tensor("shard_buf", [shard_pad, H], F16).ap()
    tables = [
        nc.dram_tensor(f"table{i}", [tabrows, 128], F16, addr_space="Shared").ap()
        for i in range(LAYERS)
    ]

    # feature-major x_first per layer lives in DRAM (SBUF is too small);
    # gather indices / weight planes stay resident in SBUF
    xf = [nc.dram_tensor(f"xf{i}", [H, shard_pad], F32).ap() for i in range(2)]
    idx_sb = nc.alloc_sbuf_tensor("idx_sb", [128, 8 * sumK], I16).ap()
    w4_sb = nc.alloc_sbuf_tensor("w4_sb", [128, 4 * sumK], F16).ap()

    cc_sem_cm = nc.semaphore("cc_sem")
    cc_sem = cc_sem_cm.__enter__()

    # ---------------- TC1: h1 = relu(x @ W1 + b1) for own shard ----------
    with tile.TileContext(nc) as tc:
        with (
            tc.tile_pool(name="cp", bufs=2) as cp,
            tc.tile_pool(name="sp", bufs=2) as sp,
            tc.tile_pool(name="const1", bufs=1) as cst,
            tc.tile_pool(name="pp", bufs=2, space="PSUM") as pp,
        ):
            ident32 = cst.tile([H, H], F32)
            make_identity(nc, ident32[:])
            w1t = cst.tile([H + 1, H], F32)
            nc.sync.dma_start(out=w1t[:], in_=w1_d[:])
            nc.sync.dma_start(out=idx_sb[:], in_=idx_d[:])
            nc.sync.dma_start(out=w4_sb[:], in_=w4_d[:])
            for b in range(batches):
                cols = slice(512 * b, 512 * (b + 1))
                xt = cp.tile([H + 1, 512], F32)
                nc.sync.dma_start(out=xt[:], in_=xT[:, cols])
                ps = pp.tile([H, 512], F32)
                nc.tensor.matmul(out=ps[:], lhsT=w1t[:], rhs=xt[:], start=True, stop=True)
                h1b = cp.tile([H + 1, 512], F32)
                nc.scalar.activation(h1b[0:H, :], ps[:], AF.Relu)
                nc.sync.dma_start(out=xf[0][:, cols], in_=h1b[0:H, :])
                tp = pp.tile([128, 128], F32)
                for j in range(4):
                    nc.tensor.transpose(
                        out=tp[:, 32 * j : 32 * (j + 1)],
                        in_=h1b[0:H, 128 * j : 128 * (j + 1)],
                        identity=ident32[:],
                    )
                hfp = sp.tile([128, 128], F16)
                nc.vector.tensor_copy(out=hfp[:], in_=tp[:])
                nc.sync.dma_start(
                    out=shard_buf[cols, :].rearrange("(j q) f -> q j f", q=128),
                    in_=hfp[:],
                )

    rg = [list(range(NCORES))]
    nc.all_engine_barrier()
    nc.gpsimd.collective_compute(
        "AllGather", ALU.bypass, replica_groups=rg,
        ins=[shard_buf.rearrange("a b -> (a b)")],
        outs=[tables[0].rearrange("a b -> (a b)")],
    ).then_inc(cc_sem, 1)
    nc.gpsimd.wait_ge(cc_sem, 1)
    nc.all_engine_barrier()

    # ---------------- layers ----------------
    call_q = [0]

    def build_layer(li, b0, b1):
        last = li == LAYERS - 1
        col0i = np.zeros(panels + 1, dtype=np.int64)
        col0i[1:] = np.cumsum(8 * K_uni)
        col0w = np.zeros(panels + 1, dtype=np.int64)
        col0w[1:] = np.cumsum(4 * K_uni)
        with tile.TileContext(nc) as tc:
            with (
                tc.tile_pool(name="gp", bufs=4) as gp,
                tc.tile_pool(name="mp", bufs=2) as mp,
                tc.tile_pool(name="ap_", bufs=3) as apo,
                tc.tile_pool(name="a33", bufs=3) as a33,
                tc.tile_pool(name="sp", bufs=2) as sp,
                tc.tile_pool(name="const2", bufs=1) as cst,
                tc.tile_pool(name="pp", bufs=1, space="PSUM") as pp,
                tc.tile_pool(name="pg", bufs=2, space="PSUM") as pg,
            ):
                ident = cst.tile([128, 128], F32)
                make_identity(nc, ident[:])
                ident32 = cst.tile([H, H], F32)
                make_identity(nc, ident32[:])
                wnn_t = cst.tile([H + 1, H], F32)
                nc.sync.dma_start(
                    out=wnn_t[:], in_=wnn_d[li * (H + 1) : (li + 1) * (H + 1), :]
                )
                wih_t = cst.tile([H + 1, 3 * H], F32)
                nc.sync.dma_start(out=wih_t[:], in_=wih_d[:])
                whh_t = cst.tile([H + 1, 3 * H], F32)
                nc.sync.dma_start(out=whh_t[:], in_=whh_d[:])
                if last:
                    wout_t = cst.tile([H + 1, NCLS], F32)
                    nc.sync.dma_start(out=wout_t[:], in_=wout_d[:])
                    npan = 4 * (b1 - b0)
                    lg_sb = cst.tile([128, NCLS * npan], F32)

                table = tables[li]
                for b in range(b0, b1):
                    cols = slice(512 * b, 512 * (b + 1))
                    agg33 = a33.tile([H + 1, 512], F32)
                    nc.vector.memset(agg33[H : H + 1, :], 1.0)
                    for pj in range(4):
                        p = 4 * b + pj
                        K = int(K_uni[p])
                        pc = slice(128 * pj, 128 * (pj + 1))
                        if K == 0:
                            nc.vector.memset(agg33[0:H, pc], 0.0)
                            continue
                        accum = apo.tile([128, H], F32)
                        for a in range(-(-K // KCH)):
                            kk = min(KCH, K - KCH * a)
                            gt = gp.tile([128, KCH, 128], F16)
                            ic = int(col0i[p]) + 64 * a
                            nc.gpsimd.dma_gather(
                                out_ap=gt[:, :kk, :],
                                in_ap=table[:],
                                idxs_ap=idx_sb[:, ic : ic + 8 * kk],
                                num_idxs=128 * kk,
                                num_idxs_reg=128 * kk,
                                elem_size=128,
                                queue_num=call_q[0] % 4,
                            )
                            call_q[0] += 1
                            msg = mp.tile([128, 4 * KCH, H], F32)
                            wc = int(col0w[p]) + 4 * KCH * a
                            nc.vector.tensor_tensor(
                                out=msg[:, : 4 * kk, :],
                                in0=gt[:, :kk, :].rearrange(
                                    "p k (j f) -> p (k j) f", j=4
                                ),
                                in1=w4_sb[:, wc : wc + 4 * kk, None].to_broadcast(
                                    [128, 4 * kk, H]
                                ),
                                op=ALU.mult,
                            )
                            red_in = msg[:, : 4 * kk, :].rearrange("p b f -> p f b")
                            if a == 0:
                                nc.vector.tensor_reduce(
                                    out=accum[:], in_=red_in,
                                    axis=mybir.AxisListType.X, op=ALU.add,
                                )
                            else:
                                tmp = apo.tile([128, H], F32)
                                nc.vector.tensor_reduce(
                                    out=tmp[:], in_=red_in,
                                    axis=mybir.AxisListType.X, op=ALU.add,
                                )
                                nc.vector.tensor_add(
                                    out=accum[:], in0=accum[:], in1=tmp[:]
                                )
                        tpp = pg.tile([H, 128], F32)
                        nc.tensor.transpose(out=tpp[:], in_=accum[:], identity=ident[:])
                        nc.vector.tensor_copy(out=agg33[0:H, pc], in_=tpp[:])

                    # ---- node phase (feature-major, bias via ones row) ----
                    ps1 = pp.tile([H, 512], F32)
                    nc.tensor.matmul(out=ps1[:], lhsT=wnn_t[:], rhs=agg33[:], start=True, stop=True)
                    oi33 = a33.tile([H + 1, 512], F32)
                    nc.vector.memset(oi33[H : H + 1, :], 1.0)
                    nc.vector.tensor_copy(out=oi33[0:H, :], in_=ps1[:])
                    gip = pp.tile([3 * H, 512], F32)
                    nc.tensor.matmul(out=gip[:], lhsT=wih_t[:], rhs=oi33[:], start=True, stop=True)
                    xfb = a33.tile([H + 1, 512], F32)
                    nc.vector.memset(xfb[H : H + 1, :], 1.0)
                    nc.sync.dma_start(out=xfb[0:H, :], in_=xf[li][:, cols])
                    ghp = pp.tile([3 * H, 512], F32)
                    nc.tensor.matmul(out=ghp[:], lhsT=whh_t[:], rhs=xfb[:], start=True, stop=True)
                    # DVE can read one PSUM operand and needs equal SB base
                    # partitions, so stage each gate slice at partition 0
                    gir = sp.tile([H, 512], F32)
                    gin = sp.tile([H, 512], F32)
                    ghz = sp.tile([H, 512], F32)
                    ghn = sp.tile([H, 512], F32)
                    nc.vector.tensor_copy(out=gir[:], in_=gip[0:H, :])
                    nc.vector.tensor_copy(out=gin[:], in_=gip[2 * H : 3 * H, :])
                    nc.vector.tensor_copy(out=ghz[:], in_=ghp[H : 2 * H, :])
                    nc.vector.tensor_copy(out=ghn[:], in_=ghp[2 * H : 3 * H, :])

                    r_t = sp.tile([H, 512], F32)
                    z_t = sp.tile([H, 512], F32)
                    n_t = sp.tile([H, 512], F32)
                    t1 = sp.tile([H, 512], F32)
                    nc.vector.tensor_add(out=r_t[:], in0=gir[:], in1=ghp[0:H, :])
                    nc.scalar.activation(r_t[:], r_t[:], AF.Sigmoid)
                    nc.vector.tensor_add(out=z_t[:], in0=ghz[:], in1=gip[H : 2 * H, :])
                    nc.scalar.activation(z_t[:], z_t[:], AF.Sigmoid)
                    nc.vector.tensor_mul(out=n_t[:], in0=r_t[:], in1=ghn[:])
                    nc.vector.tensor_add(out=n_t[:], in0=n_t[:], in1=gin[:])
                    nc.scalar.activation(n_t[:], n_t[:], AF.Tanh)
                    # h_new = n + z*(xf - n);  h_out = h_new + fuse*xf
                    nc.vector.tensor_sub(out=t1[:], in0=xfb[0:H, :], in1=n_t[:])
                    nc.vector.tensor_mul(out=t1[:], in0=t1[:], in1=z_t[:])
                    nc.vector.tensor_add(out=t1[:], in0=t1[:], in1=n_t[:])
                    ho33 = a33.tile([H + 1, 512], F32)
                    nc.vector.memset(ho33[H : H + 1, :], 1.0)
                    nc.vector.tensor_scalar(
                        out=ho33[0:H, :], in0=xfb[0:H, :],
                        scalar1=float(fuse[li]), scalar2=None, op0=ALU.mult,
                    )
                    nc.vector.tensor_add(out=ho33[0:H, :], in0=ho33[0:H, :], in1=t1[:])

                    if not last:
                        nc.sync.dma_start(out=xf[li + 1][:, cols], in_=ho33[0:H, :])
                        tp = pp.tile([128, 128], F32)
                        for j in range(4):
                            nc.tensor.transpose(
                                out=tp[:, 32 * j : 32 * (j + 1)],
                                in_=ho33[0:H, 128 * j : 128 * (j + 1)],
                                identity=ident32[:],
                            )
                        hfp = sp.tile([128, 128], F16)
                        nc.vector.tensor_copy(out=hfp[:], in_=tp[:])
                        nc.sync.dma_start(
                            out=shard_buf[cols, :].rearrange("(j q) f -> q j f", q=128),
                            in_=hfp[:],
                        )
                    else:
                        lps = pg.tile([128, 4 * NCLS], F32)
                        for j in range(4):
                            nc.tensor.matmul(
                                out=lps[:, NCLS * j : NCLS * (j + 1)],
                                lhsT=ho33[:, 128 * j : 128 * (j + 1)],
                                rhs=wout_t[:], start=True, stop=True,
                            )
                        nc.vector.tensor_copy(
                            out=lg_sb[:, NCLS * 4 * (b - b0) : NCLS * 4 * (b - b0 + 1)],
                            in_=lps[:],
                        )

                if last:
                    # log_softmax over the 2 classes, node-major [128, npan, 2]
                    lg = lg_sb[:].rearrange("p (n c) -> p n c", c=NCLS)
                    mx = sp.tile([128, npan], F32)
                    nc.vector.tensor_reduce(
                        out=mx[:], in_=lg, axis=mybir.AxisListType.X, op=ALU.max
                    )
                    df = sp.tile([128, npan, NCLS], F32)
                    nc.vector.tensor_tensor(
                        out=df[:],
                        in0=lg,
                        in1=mx[:, :, None].to_broadcast([128, npan, NCLS]),
                        op=ALU.subtract,
                    )
                    ex = sp.tile([128, npan, NCLS], F32)
                    nc.scalar.activation(ex[:], df[:], AF.Exp)
                    sm = sp.tile([128, npan], F32)
                    nc.vector.tensor_reduce(
                        out=sm[:], in_=ex[:], axis=mybir.AxisListType.X, op=ALU.add
                    )
                    nc.scalar.activation(sm[:], sm[:], AF.Ln)
                    ou = sp.tile([128, npan, NCLS], F32)
                    nc.vector.tensor_tensor(
                        out=ou[:],
                        in0=df[:],
                        in1=sm[:, :, None].to_broadcast([128, npan, NCLS]),
                        op=ALU.subtract,
                    )
                    nc.sync.dma_start(
                        out=out_d[:, NCLS * 4 * b0 : NCLS * 4 * b1],
                        in_=ou[:].rearrange("p n c -> p (n c)"),
                    )

    bs = sorted(set([0, batches // 3, 2 * batches // 3, batches]))
    for i in range(len(bs) - 1):
        build_layer(0, bs[i], bs[i + 1])
    nc.all_engine_barrier()
    nc.gpsimd.collective_compute(
        "AllGather", ALU.bypass, replica_groups=rg,
        ins=[shard_buf.rearrange("a b -> (a b)")],
        outs=[tables[1].rearrange("a b -> (a b)")],
    ).then_inc(cc_sem, 1)
    nc.gpsimd.wait_ge(cc_sem, 2)
    nc.all_engine_barrier()
    for i in range(len(bs) - 1):
        build_layer(1, bs[i], bs[i + 1])

    nc.compile()
    _split_multiwaits(nc)
    cc_sem_cm.__exit__(None, None, None)
    _BUILD_CACHE[key] = nc
    return nc


def _prepare(x, edge_index, edge_weight, W_first, b_first, W_nn, b_nn,
             W_ih, b_ih, W_hh, b_hh, fuse_weight, W_out, b_out):
    shard, shard_pad, panels, tabrows = _sizes(N)
    pre = _preprocess(edge_index, edge_weight)
    order = pre["order"]
    fuse = np.asarray(fuse_weight, np.float32)

    nc = _build(pre["K_uni"], fuse)

    x = np.asarray(x, np.float32)
    w1 = _w33(np.asarray(W_first, np.float32).T, b_first)
    wnn = np.concatenate(
        [_w33(np.asarray(W_nn[i], np.float32).T, b_nn[i]) for i in range(LAYERS)], 0
    )
    wih = _w33(np.asarray(W_ih, np.float32).T, b_ih)
    whh = _w33(np.asarray(W_hh, np.float32).T, b_hh)
    wout = _w33(np.asarray(W_out, np.float32).T, b_out)

    in_maps = []
    for c in range(NCORES):
        ids = order[c * shard : (c + 1) * shard]
        xs = np.zeros((H + 1, shard_pad), np.float32)
        xs[0:H, 0:shard] = x[ids].T
        xs[H, :] = 1.0
        in_maps.append(
            {
                "xT": xs,
                "idx": pre["idx_imgs"][c],
                "w4": pre["w4_imgs"][c],
                "w1": w1,
                "wnn": wnn,
                "wih": wih,
                "whh": whh,
                "wout": wout,
            }
        )

    return nc, in_maps, order


def _assemble(order, results):
    shard, shard_pad, panels, tabrows = _sizes(N)
    out = np.zeros((N, NCLS), np.float32)
    for c in range(NCORES):
        R = np.asarray(results[c]["out"])  # [128, 2*panels]
        R = R.reshape(128, panels, NCLS).transpose(1, 0, 2).reshape(-1, NCLS)
        ids = order[c * shard : (c + 1) * shard]
        out[ids] = R[0:shard]
    return out


def kernel(**inputs):
    nc, in_maps, order = _prepare(**inputs)
    res = run_bass_kernel_spmd(nc, in_maps, core_ids=list(range(NCORES)))
    return _assemble(order, res.results)



# revision 5
# speedup vs baseline: 2.1963x; 2.1963x over previous
"""GatedGCN Trainium2 kernel — 8-core SPMD, self-contained. v2

Strategy
--------
dst-shard the graph across 8 NeuronCores. Node features live in DRAM as an
fp16 table with 4 nodes packed per 256B row, so the bulk `dma_gather`
(int16 indices, 256B elements) can fetch `h[src]` for every edge. Nodes are
relabeled per shard by descending in-degree and grouped into 128-node
panels; each edge occupies a (node q, slot k) cell of the panel grid, so
the gather tile's partition dim is already dst-aligned.

Per slot-column pipeline (no per-edge scatter anywhere):
  gather gt[128q, kk, 128(4j*32f)]  (SWDGE, 1024-idx calls, 4 queues)
  msg = gt * w32 (DVE f16 2x; w32 = host-expanded per-slot weight planes
        streamed from DRAM over HWDGE)
  per slot column k: matmul(psP[128(4j*32f), 128q] += msg[:,k,:]^T,
        lhsT=msg column, rhs=identity)  -- TensorE transpose-accumulate
        performs the whole weighted segment-sum in PSUM, feature-major.
The 4-way sub-row packing is folded into the node-phase matmul by tiling
W_nn^T 4x on the contraction axis (wnn4), so the packed lanes sum for free.
GRU biases ride on ScalarE activations (per-partition bias APs); gi+gh for
the r/z gates accumulate in one PSUM tile. Layer hand-off between cores is
an fp16 AllGather of the packed table. log_softmax + b_out on host (shift
invariance makes that exact).
"""
import sys

sys.path.insert(0, "/opt/trn_rl_repo")

import numpy as np

import concourse.bacc as bacc
import concourse.bass as bass
import concourse.mybir as mybir
import concourse.tile as tile
from concourse.bass_utils import run_bass_kernel_spmd
from concourse.masks import make_identity

N = 100000
E = 1600000
H = 32
NCLS = 2
LAYERS = 2
NCORES = 8
KCH = 8  # slots per dma_gather call = 128*KCH idxs (1024-idx ring cap)

F32 = mybir.dt.float32
F16 = mybir.dt.float16
I16 = mybir.dt.int16
AF = mybir.ActivationFunctionType
ALU = mybir.AluOpType


def _split_multiwaits(nc, max_waits=1):
    """This walrus build rejects >1 sync-wait per instruction; split extras
    onto same-engine InstNoOp predecessors (semantically identical)."""
    ctr = 0
    for fn in nc.m.functions:
        for bb in fn.blocks:
            new_insts = []
            for inst in bb.instructions:
                si = inst.sync_info
                waits = list(si.on_wait) if si is not None and si.on_wait else []
                if len(waits) > max_waits:
                    head, tail = waits[:-max_waits], waits[-max_waits:]
                    for i in range(0, len(head), max_waits):
                        ctr += 1
                        nop = mybir.InstNoOp(name=f"WSPLIT-{ctr}", engine=inst.engine)
                        nop.sync_info = mybir.SyncInfo(
                            on_wait=head[i : i + max_waits], on_update=[]
                        )
                        nc.register_instruction(nop, overwrite=True)
                        new_insts.append(nop)
                    inst.sync_info = mybir.SyncInfo(
                        on_wait=tail,
                        on_update=list(si.on_update) if si.on_update else [],
                    )
                new_insts.append(inst)
            bb.instructions[:] = new_insts


def _sizes(n):
    shard = n // NCORES
    shard_pad = -(-shard // 512) * 512
    panels = shard_pad // 128
    tabrows = NCORES * shard_pad // 4
    return shard, shard_pad, panels, tabrows


def _preprocess(edge_index, edge_weight):
    shard, shard_pad, panels, tabrows = _sizes(N)
    src_ = np.asarray(edge_index[0], dtype=np.int64)
    dst = np.asarray(edge_index[1], dtype=np.int64)
    src = src_
    w = np.asarray(edge_weight, dtype=np.float32)

    deg = np.bincount(dst, minlength=N)
    shards = np.arange(N) // shard
    order = np.lexsort((np.arange(N), -deg, shards))  # old ids by (shard, -deg)
    new_of_old = np.empty(N, dtype=np.int64)
    pos = np.arange(N)
    c_of_pos = pos // shard
    r_of_pos = pos - c_of_pos * shard
    new_of_old[order] = c_of_pos * shard_pad + r_of_pos

    s_new = new_of_old[src]
    d_new = new_of_old[dst]
    core = d_new // shard_pad
    r = d_new % shard_pad
    # slot index k per edge: occurrence number among edges sharing the dst
    eorder = np.argsort(d_new, kind="stable")
    ds = d_new[eorder]
    starts = np.r_[0, np.nonzero(np.diff(ds))[0] + 1]
    counts = np.diff(np.r_[starts, len(ds)])
    k_sorted = np.arange(len(ds)) - np.repeat(starts, counts)
    k = np.empty(src_.size, dtype=np.int64)
    k[eorder] = k_sorted

    # per-core per-panel K, unified across cores (SPMD: one program)
    deg_new = np.zeros(NCORES * shard_pad, dtype=np.int64)
    deg_new[new_of_old] = deg
    K_uni = np.zeros(panels, dtype=np.int64)
    for c in range(NCORES):
        base = c * shard_pad
        firsts = deg_new[base : base + shard_pad : 128]  # max of each panel
        K_uni = np.maximum(K_uni, firsts)
    K_uni = K_uni.astype(np.int64)
    sumK = int(K_uni.sum())
    col0 = np.zeros(panels + 1, dtype=np.int64)
    col0[1:] = np.cumsum(128 * K_uni)
    slots_total = int(col0[-1])

    p_of_edge = r // 128
    q_of_edge = r % 128
    slotpos = col0[p_of_edge] + k * 128 + q_of_edge

    idx_imgs = np.zeros((NCORES, 128, 8 * sumK), dtype=np.int16)
    w32_imgs = np.zeros((NCORES, 128, sumK * 128), dtype=np.float16)
    for c in range(NCORES):
        m = core == c
        ia = np.zeros(slots_total, dtype=np.int16)
        wa = np.zeros(slots_total * 4, dtype=np.float16)
        ia[slotpos[m]] = (s_new[m] >> 2).astype(np.int16)
        wa[slotpos[m] * 4 + (s_new[m] & 3)] = w[m].astype(np.float16)
        icols = 0
        wcols = 0
        for p in range(panels):
            K = int(K_uni[p])
            if K == 0:
                continue
            a, b = int(col0[p]), int(col0[p + 1])
            blk = ia[a:b].reshape(K * 8, 16).T  # [16, 8K]
            idx_imgs[c, :, icols : icols + 8 * K] = np.tile(blk, (8, 1))
            # w4 per panel: [128 q, K, 4 j] -> expand along f (32) so the DVE
            # multiply is dense step-1 fp16 (2x mode)
            wb = wa[4 * a : 4 * b].reshape(K, 128, 4)  # [K, q, j]
            w4 = wb.transpose(1, 0, 2)  # [q, K, j]
            w32 = np.repeat(w4.reshape(128, K, 4, 1), H, axis=3)  # [q,K,4,32]
            w32_imgs[c, :, wcols : wcols + 128 * K] = w32.reshape(128, 128 * K)
            icols += 8 * K
            wcols += 128 * K
    return {
        "order": order,
        "K_uni": K_uni,
        "idx_imgs": idx_imgs,
        "w32_imgs": w32_imgs,
        "sumK": sumK,
    }


_BUILD_CACHE = {}


def _build(K_uni, fuse):
    key = (tuple(int(x) for x in K_uni), tuple(float(x) for x in fuse))
    if key in _BUILD_CACHE:
        return _BUILD_CACHE[key]

    shard, shard_pad, panels, tabrows = _sizes(N)
    sumK = int(np.sum(K_uni))
    batches = panels // 4

    nc = bacc.Bacc(
        "TRN2",
        target_bir_lowering=False,
        debug=False,
        num_devices=NCORES,
        num_swdge_queues=4,
        dynamic_dma_scratch_size=32768,
    )
    xT = nc.dram_tensor("xT", [H, shard_pad], F16, kind="ExternalInput").ap()
    idx_d = nc.dram_tensor("idx", [128, 8 * sumK], I16, kind="ExternalInput").ap()
    w32_d = nc.dram_tensor("w32", [128, sumK * 128], F16, kind="ExternalInput").ap()
    # weights, feature-major lhsT layouts (f16)
    w1_d = nc.dram_tensor("w1", [H, H], F16, kind="ExternalInput").ap()
    wnn4_d = nc.dram_tensor("wnn4", [LAYERS * 128, H], F16, kind="ExternalInput").ap()
    wih_d = nc.dram_tensor("wih", [H, 3 * H], F16, kind="ExternalInput").ap()
    whh_d = nc.dram_tensor("whh", [H, 3 * H], F16, kind="ExternalInput").ap()
    wout_d = nc.dram_tensor("wout", [H, NCLS], F16, kind="ExternalInput").ap()
    # bias columns [*, 1] f32
    b1_d = nc.dram_tensor("b1", [H, 1], F32, kind="ExternalInput").ap()
    bnn_d = nc.dram_tensor("bnn", [LAYERS * H, 1], F32, kind="ExternalInput").ap()
    brz_d = nc.dram_tensor("brz", [2 * H, 1], F32, kind="ExternalInput").ap()
    binn_d = nc.dram_tensor("binn", [H, 1], F32, kind="ExternalInput").ap()
    bhn_d = nc.dram_tensor("bhn", [H, 1], F32, kind="ExternalInput").ap()
    out_d = nc.dram_tensor("out", [128, NCLS * panels], F32, kind="ExternalOutput").ap()

    shard_buf = nc.dram_tensor("shard_buf", [shard_pad, H], F16).ap()
    tables = [
        nc.dram_tensor(f"table{i}", [tabrows, 128], F16, addr_space="Shared").ap()
        for i in range(LAYERS)
    ]
    xf = [nc.dram_tensor(f"xf{i}", [H, shard_pad], F16).ap() for i in range(2)]
    idx_sb = nc.alloc_sbuf_tensor("idx_sb", [128, 8 * sumK], I16).ap()

    cc_sem_cm = nc.semaphore("cc_sem")
    cc_sem = cc_sem_cm.__enter__()

    col0i = np.zeros(panels + 1, dtype=np.int64)
    col0i[1:] = np.cumsum(8 * K_uni)
    col0w = np.zeros(panels + 1, dtype=np.int64)
    col0w[1:] = np.cumsum(128 * K_uni)

    # ---------------- TC1: h1 = relu(x @ W1 + b1) for own shard ----------
    with tile.TileContext(nc) as tc:
        with (
            tc.tile_pool(name="cp", bufs=2) as cp,
            tc.tile_pool(name="sp", bufs=2) as sp,
            tc.tile_pool(name="const1", bufs=1) as cst,
            tc.tile_pool(name="pp", bufs=2, space="PSUM") as pp,
        ):
            ident32 = cst.tile([H, H], F16)
            make_identity(nc, ident32[:])
            w1t = cst.tile([H, H], F16)
            nc.sync.dma_start(out=w1t[:], in_=w1_d[:])
            b1t = cst.tile([H, 1], F32)
            nc.sync.dma_start(out=b1t[:], in_=b1_d[:])
            nc.sync.dma_start(out=idx_sb[:], in_=idx_d[:])
            for b in range(batches):
                cols = slice(512 * b, 512 * (b + 1))
                xt = cp.tile([H, 512], F16)
                nc.sync.dma_start(out=xt[:], in_=xT[:, cols])
                ps = pp.tile([H, 512], F32)
                nc.tensor.matmul(out=ps[:], lhsT=w1t[:], rhs=xt[:], start=True, stop=True)
                h1b = cp.tile([H, 512], F16)
                nc.scalar.activation(h1b[:], ps[:], AF.Relu, bias=b1t[:])
                nc.sync.dma_start(out=xf[0][:, cols], in_=h1b[:])
                tp = pp.tile([128, 128], F16)
                for j in range(4):
                    nc.tensor.transpose(
                        out=tp[:, 32 * j : 32 * (j + 1)],
                        in_=h1b[:, 128 * j : 128 * (j + 1)],
                        identity=ident32[:],
                    )
                hfp = sp.tile([128, 128], F16)
                nc.scalar.activation(hfp[:], tp[:], AF.Copy)
                nc.sync.dma_start(
                    out=shard_buf[cols, :].rearrange("(j q) f -> q j f", q=128),
                    in_=hfp[:],
                )

    rg = [list(range(NCORES))]
    nc.all_engine_barrier()
    nc.gpsimd.collective_compute(
        "AllGather", ALU.bypass, replica_groups=rg,
        ins=[shard_buf.rearrange("a b -> (a b)")],
        outs=[tables[0].rearrange("a b -> (a b)")],
    ).then_inc(cc_sem, 1)
    nc.gpsimd.wait_ge(cc_sem, 1)
    nc.all_engine_barrier()

    # ---------------- layers ----------------
    call_q = [0]

    def build_layer(li):
        last = li == LAYERS - 1
        with tile.TileContext(nc) as tc:
            with (
                tc.tile_pool(name="gp", bufs=8) as gp,
                tc.tile_pool(name="wp", bufs=4) as wp,
                tc.tile_pool(name="mp", bufs=4) as mp,
                tc.tile_pool(name="ap_", bufs=2) as apo,
                tc.tile_pool(name="sp", bufs=3) as sp,
                tc.tile_pool(name="const2", bufs=1) as cst,
                tc.tile_pool(name="pg", bufs=2, space="PSUM") as pg,
                tc.tile_pool(name="pn", bufs=1, space="PSUM") as pn,
            ):
                ident = cst.tile([128, 128], F16)
                make_identity(nc, ident[:])
                wnn4_t = cst.tile([128, H], F16)
                nc.sync.dma_start(
                    out=wnn4_t[:], in_=wnn4_d[li * 128 : (li + 1) * 128, :]
                )
                wih_t = cst.tile([H, 3 * H], F16)
                nc.sync.dma_start(out=wih_t[:], in_=wih_d[:])
                whh_t = cst.tile([H, 3 * H], F16)
                nc.sync.dma_start(out=whh_t[:], in_=whh_d[:])
                bnn_t = cst.tile([H, 1], F32)
                nc.sync.dma_start(out=bnn_t[:], in_=bnn_d[li * H : (li + 1) * H, :])
                br_t = cst.tile([H, 1], F32)
                nc.sync.dma_start(out=br_t[:], in_=brz_d[0:H, :])
                bz_t = cst.tile([H, 1], F32)
                nc.sync.dma_start(out=bz_t[:], in_=brz_d[H : 2 * H, :])
                binn_t = cst.tile([H, 1], F32)
                nc.sync.dma_start(out=binn_t[:], in_=binn_d[:])
                bhn_t = cst.tile([H, 1], F32)
                nc.sync.dma_start(out=bhn_t[:], in_=bhn_d[:])
                if last:
                    wout_t = cst.tile([H, NCLS], F16)
                    nc.sync.dma_start(out=wout_t[:], in_=wout_d[:])
                    lg_sb = cst.tile([128, NCLS * panels], F32)
                if not last:
                    ident32 = cst.tile([H, H], F16)
                    make_identity(nc, ident32[:])

                table = tables[li]
                for b in range(batches):
                    cols = slice(512 * b, 512 * (b + 1))
                    aggF = apo.tile([128, 512], F16)
                    for pj in range(4):
                        p = 4 * b + pj
                        K = int(K_uni[p])
                        pc = slice(128 * pj, 128 * (pj + 1))
                        if K == 0:
                            nc.vector.memset(aggF[:, pc], 0.0)
                            continue
                        psP = pg.tile([128, 128], F32)
                        ncalls = -(-K // KCH)
                        for a in range(ncalls):
                            kk = min(KCH, K - KCH * a)
                            gt = gp.tile([128, KCH, 128], F16)
                            ic = int(col0i[p]) + 8 * KCH * a
                            nc.gpsimd.dma_gather(
                                out_ap=gt[:, :kk, :],
                                in_ap=table[:],
                                idxs_ap=idx_sb[:, ic : ic + 8 * kk],
                                num_idxs=128 * kk,
                                num_idxs_reg=128 * kk,
                                elem_size=128,
                                queue_num=call_q[0] % 4,
                            )
                            call_q[0] += 1
                            wc = int(col0w[p]) + 128 * KCH * a
                            w32t = wp.tile([128, KCH, 128], F16)
                            nc.sync.dma_start(
                                out=w32t[:, :kk, :],
                                in_=w32_d[:, wc : wc + 128 * kk].rearrange(
                                    "p (k e) -> p k e", e=128
                                ),
                            )
                            msg = mp.tile([128, KCH, 128], F16)
                            nc.vector.tensor_tensor(
                                out=msg[:, :kk, :], in0=gt[:, :kk, :],
                                in1=w32t[:, :kk, :], op=ALU.mult,
                            )
                            for k in range(kk):
                                nc.tensor.matmul(
                                    out=psP[:],
                                    lhsT=msg[:, k, :],
                                    rhs=ident[:],
                                    start=(a == 0 and k == 0),
                                    stop=(a == ncalls - 1 and k == kk - 1),
                                )
                        nc.scalar.activation(aggF[:, pc], psP[:], AF.Copy)

                    # ---- node phase (feature-major; biases on ScalarE) ----
                    ps1 = pn.tile([H, 512], F32)
                    nc.tensor.matmul(out=ps1[:], lhsT=wnn4_t[:], rhs=aggF[:], start=True, stop=True)
                    oi = sp.tile([H, 512], F16)
                    nc.scalar.activation(oi[:], ps1[:], AF.Identity, bias=bnn_t[:])
                    xfb = sp.tile([H, 512], F16)
                    nc.sync.dma_start(out=xfb[:], in_=xf[li][:, cols])
                    ps_rz = pn.tile([2 * H, 512], F32)
                    nc.tensor.matmul(out=ps_rz[:], lhsT=wih_t[:, 0 : 2 * H], rhs=oi[:], start=True, stop=False)
                    nc.tensor.matmul(out=ps_rz[:], lhsT=whh_t[:, 0 : 2 * H], rhs=xfb[:], start=False, stop=True)
                    ps_n1 = pn.tile([H, 512], F32)
                    nc.tensor.matmul(out=ps_n1[:], lhsT=wih_t[:, 2 * H : 3 * H], rhs=oi[:], start=True, stop=True)
                    ps_n2 = pn.tile([H, 512], F32)
                    nc.tensor.matmul(out=ps_n2[:], lhsT=whh_t[:, 2 * H : 3 * H], rhs=xfb[:], start=True, stop=True)
                    r_t = sp.tile([H, 512], F16)
                    nc.scalar.activation(r_t[:], ps_rz[0:H, :], AF.Sigmoid, bias=br_t[:])
                    z_t = sp.tile([H, 512], F16)
                    nc.scalar.activation(z_t[:], ps_rz[H : 2 * H, :], AF.Sigmoid, bias=bz_t[:])
                    inb = sp.tile([H, 512], F16)
                    nc.scalar.activation(inb[:], ps_n1[:], AF.Identity, bias=binn_t[:])
                    hn = sp.tile([H, 512], F16)
                    nc.scalar.activation(hn[:], ps_n2[:], AF.Identity, bias=bhn_t[:])
                    t1 = sp.tile([H, 512], F16)
                    nc.vector.tensor_mul(out=t1[:], in0=r_t[:], in1=hn[:])
                    nc.vector.tensor_add(out=t1[:], in0=t1[:], in1=inb[:])
                    n_t = sp.tile([H, 512], F16)
                    nc.scalar.activation(n_t[:], t1[:], AF.Tanh)
                    # h' = n + z*(xf - n);  ho = h' + fuse*xf
                    t2 = sp.tile([H, 512], F16)
                    nc.vector.tensor_sub(out=t2[:], in0=xfb[:], in1=n_t[:])
                    nc.vector.tensor_mul(out=t2[:], in0=t2[:], in1=z_t[:])
                    nc.vector.tensor_add(out=t2[:], in0=t2[:], in1=n_t[:])
                    ho = sp.tile([H, 512], F16)
                    nc.vector.scalar_tensor_tensor(
                        out=ho[:], in0=xfb[:], scalar=float(fuse[li]), in1=t2[:],
                        op0=ALU.mult, op1=ALU.add,
                    )

                    if not last:
                        nc.sync.dma_start(out=xf[li + 1][:, cols], in_=ho[:])
                        tp = pg.tile([128, 128], F16)
                        for j in range(4):
                            nc.tensor.transpose(
                                out=tp[:, 32 * j : 32 * (j + 1)],
                                in_=ho[:, 128 * j : 128 * (j + 1)],
                                identity=ident32[:],
                            )
                        hfp = sp.tile([128, 128], F16)
                        nc.scalar.activation(hfp[:], tp[:], AF.Copy)
                        nc.sync.dma_start(
                            out=shard_buf[cols, :].rearrange("(j q) f -> q j f", q=128),
                            in_=hfp[:],
                        )
                    else:
                        lps = pg.tile([128, 4 * NCLS], F32)
                        for j in range(4):
                            nc.tensor.matmul(
                                out=lps[:, NCLS * j : NCLS * (j + 1)],
                                lhsT=ho[:, 128 * j : 128 * (j + 1)],
                                rhs=wout_t[:], start=True, stop=True,
                            )
                        nc.scalar.activation(
                            lg_sb[:, NCLS * 4 * b : NCLS * 4 * (b + 1)], lps[:],
                            AF.Copy,
                        )

                if last:
                    nc.sync.dma_start(out=out_d[:], in_=lg_sb[:])

    build_layer(0)
    nc.all_engine_barrier()
    nc.gpsimd.collective_compute(
        "AllGather", ALU.bypass, replica_groups=rg,
        ins=[shard_buf.rearrange("a b -> (a b)")],
        outs=[tables[1].rearrange("a b -> (a b)")],
    ).then_inc(cc_sem, 1)
    nc.gpsimd.wait_ge(cc_sem, 2)
    nc.all_engine_barrier()
    build_layer(1)

    nc.compile()
    _split_multiwaits(nc)
    cc_sem_cm.__exit__(None, None, None)
    _BUILD_CACHE[key] = nc
    return nc


def _prepare(x, edge_index, edge_weight, W_first, b_first, W_nn, b_nn,
             W_ih, b_ih, W_hh, b_hh, fuse_weight, W_out, b_out):
    shard, shard_pad, panels, tabrows = _sizes(N)
    pre = _preprocess(edge_index, edge_weight)
    order = pre["order"]
    fuse = np.asarray(fuse_weight, np.float32)

    nc = _build(pre["K_uni"], fuse)

    x = np.asarray(x, np.float32)
    f16 = np.float16
    w1 = np.asarray(W_first, np.float32).T.astype(f16)
    wnn4 = np.concatenate(
        [np.tile(np.asarray(W_nn[i], np.float32).T, (4, 1)) for i in range(LAYERS)], 0
    ).astype(f16)
    wihT = np.asarray(W_ih, np.float32).T
    whhT = np.asarray(W_hh, np.float32).T
    b_ih = np.asarray(b_ih, np.float32)
    b_hh = np.asarray(b_hh, np.float32)
    bnn = np.concatenate([np.asarray(b_nn[i], np.float32) for i in range(LAYERS)])
    brz = b_ih[0 : 2 * H] + b_hh[0 : 2 * H]
    binn = b_ih[2 * H : 3 * H]
    bhn = b_hh[2 * H : 3 * H]
    wout = np.asarray(W_out, np.float32).T.astype(f16)

    in_maps = []
    for c in range(NCORES):
        ids = order[c * shard : (c + 1) * shard]
        xs = np.zeros((H, shard_pad), f16)
        xs[:, 0:shard] = x[ids].T.astype(f16)
        in_maps.append(
            {
                "xT": xs,
                "idx": pre["idx_imgs"][c],
                "w32": pre["w32_imgs"][c],
                "w1": w1,
                "wnn4": wnn4,
                "wih": wihT.astype(f16),
                "whh": whhT.astype(f16),
                "wout": wout,
                "b1": np.asarray(b_first, np.float32).reshape(H, 1),
                "bnn": bnn.reshape(LAYERS * H, 1),
                "brz": brz.reshape(2 * H, 1),
                "binn": binn.reshape(H, 1),
                "bhn": bhn.reshape(H, 1),
            }
        )

    return nc, in_maps, order


def _assemble(order, results, b_out):
    shard, shard_pad, panels, tabrows = _sizes(N)
    out = np.zeros((N, NCLS), np.float64)
    for c in range(NCORES):
        R = np.asarray(results[c]["out"])  # [128, 2*panels] raw logits
        R = R.reshape(128, panels, NCLS).transpose(1, 0, 2).reshape(-1, NCLS)
        ids = order[c * shard : (c + 1) * shard]
        out[ids] = R[0:shard]
    # log_softmax(logits + b_out) on host; device logits are already
    # shift-reduced so this is exact
    out = out + np.asarray(b_out, np.float64)[None, :]
    mx = out.max(axis=1, keepdims=True)
    s = out - mx
    lse = np.log(np.exp(s).sum(axis=1, keepdims=True))
    return (s - lse).astype(np.float32)


def kernel(**inputs):
    nc, in_maps, order = _prepare(**inputs)
    res = run_bass_kernel_spmd(nc, in_maps, core_ids=list(range(NCORES)))
    return _assemble(order, res.results, inputs["b_out"])
